# revision 18
# baseline (speedup 1.0000x reference)
"""CPT attention (QKV+LoRA -> fake-quant KV -> causal attention -> proj+LoRA)
as a Bass/Tile kernel on 8 TRN2 NeuronCores.

Sharding: data parallel over batch (2) x tensor parallel over heads (16/4=4
per core), Megatron-style. Each core computes qkv for its 4 heads from the
full hidden_states[b], runs causal attention locally, and produces a partial
projection output [T, C]; the host sums the 4 tensor-parallel partials per
batch and adds b_proj.

Device-side design notes:
- All matmul operands f16 (1 col/cycle on PE vs 4 for f32), fp32 PSUM accum.
- Host pre-transposes x and all weights to f16 (no on-device load transposes).
- Scores are computed transposed (S^T[k, q]) so the softmax reduction (over k)
  is done by a ones column appended to V: the PV matmul emits the softmax
  denominator as an extra output column. No row-max subtraction needed
  (scores here are O(1)).
- Head pairs share one [128,1024] score PSUM tile: the two K=64 matmuls hit
  partition row-groups 0-63/64-127 and run concurrently in the PE array.
- fake_quant rounding uses the fp32 +/- 1.5*2^23 trick (round-to-nearest-even,
  matching jnp.round).
- Causal masking: exp, then multiply by host-provided 0/1 mask tiles (DVE).
- The whole kernel is pipelined over q-blocks: qkv-projection, V, attention,
  and output-projection for block tb interleave.
- LoRA / bias contributions are compiled out when the corresponding inputs are
  all-zero (separate cached program variants).
"""

import numpy as np

import concourse.bass as bass
import concourse.bacc as bacc
import concourse.mybir as mybir
import concourse.tile as tile
from concourse.bass_utils import run_bass_kernel_spmd

AF = mybir.ActivationFunctionType
OP = mybir.AluOpType

B, T, C = 2, 2048, 1024
H, HD = 16, 64
R = 16
ALPHA_OVER_R = 2.0
QMAX = 255.0
MAGIC = 12582912.0  # 1.5 * 2**23: fp32 add/sub rounds to nearest-even integer
N_CORES = 8
HPC = 4  # heads per core
CH = HPC * HD  # 256 channels (per each of q/k/v) per core
NT = T // 128  # 16 T-tiles
NC_ = C // 128  # 8 C-tiles
F16 = mybir.dt.float16
F32 = mybir.dt.float32


def _build_body(nc, tc, d, use_bias, use_lora_attn, use_lora_proj):
    import contextlib

    ctx = contextlib.ExitStack()
    with ctx:
        persist = ctx.enter_context(tc.tile_pool(name="persist", bufs=1))
        fqp = ctx.enter_context(tc.tile_pool(name="fqp", bufs=6))
        exp_pool = ctx.enter_context(tc.tile_pool(name="exp_pool", bufs=18))
        outp = ctx.enter_context(tc.tile_pool(name="outp", bufs=3))
        rcpp = ctx.enter_context(tc.tile_pool(name="rcpp", bufs=4))
        psS = ctx.enter_context(
            tc.tile_pool(name="psS", bufs=2, space=bass.MemorySpace.PSUM)
        )
        psB = ctx.enter_context(
            tc.tile_pool(name="psB", bufs=2, space=bass.MemorySpace.PSUM)
        )
        psV = ctx.enter_context(
            tc.tile_pool(name="psV", bufs=2, space=bass.MemorySpace.PSUM)
        )

        # ---- constants (all DMA'd; keeps GPSIMD off the startup path) ----
        consts = persist.tile([128, 4], F32, tag="consts", name="consts")
        nc.sync.dma_start(consts[:, :], d["consts"][:, :])
        inv_ap = consts[:, 0:1]
        zp_ap = consts[:, 1:2]
        sc_ap = consts[:, 2:3]
        es_ap = consts[:, 3:4]  # 0.125 * kv_scale (scores use integer-valued K)
        id16 = persist.tile([128, 128], F16, tag="id16", name="id16")
        nc.sync.dma_start(id16[:, :], d["id16"][:, :])
        maskt = persist.tile([128, 128], F16, tag="maskt", name="maskt")
        nc.sync.dma_start(maskt[:, :], d["masks"][:, :])
        if use_bias:
            ones_row = persist.tile([1, 512], F16, tag="ones_row", name="ones_row")
            nc.gpsimd.memset(ones_row[:, :], 1.0)
            bqk_row = persist.tile([1, 2 * CH], F16, tag="bqk_row", name="bqk_row")
            nc.sync.dma_start(bqk_row[:, :], d["bqk"][:, :])
            bv_row = persist.tile([1, CH], F16, tag="bv_row", name="bv_row")
            nc.sync.dma_start(bv_row[:, :], d["bv"][:, :])

        # ---- persistent f16 tensors (DMA'd pre-transposed from host) ----
        xT = [persist.tile([128, T], F16, tag=f"xT{j}", name=f"xT{j}") for j in range(NC_)]
        wqkT = [
            persist.tile([128, 2 * CH], F16, tag=f"wqkT{j}", name=f"wqkT{j}")
            for j in range(NC_)
        ]
        wvT = [
            persist.tile([128, CH], F16, tag=f"wvT{j}", name=f"wvT{j}")
            for j in range(NC_)
        ]
        wpT = [
            persist.tile([128, C], F16, tag=f"wpT{i}", name=f"wpT{i}") for i in range(2)
        ]
        qkT = [
            persist.tile([128, T], F16, tag=f"qkT{i}", name=f"qkT{i}") for i in range(4)
        ]
        Vaug = [
            persist.tile([128, HPC * (HD + 1)], F16, tag=f"Vaug{t}", name=f"Vaug{t}")
            for t in range(NT)
        ]
        attnN = [
            persist.tile([128, 4 * CH], F16, tag=f"attnN{qb}", name=f"attnN{qb}")
            for qb in range(4)
        ]  # [128, qi*256 + ch] per q-block
        attnTa = persist.tile([128, 2 * T], F16, tag="attnTa", name="attnTa")
        attnT = [attnTa[:, cb * T : (cb + 1) * T] for cb in range(2)]
        if use_lora_attn:
            AatT = [
                persist.tile([128, R], F16, tag=f"AatT{j}", name=f"AatT{j}")
                for j in range(NC_)
            ]
            BqkT = persist.tile([R, 2 * CH], F16, tag="BqkT", name="BqkT")
            BvT = persist.tile([R, CH], F16, tag="BvT", name="BvT")
            LT = persist.tile([R, T], F16, tag="LT", name="LT")
        if use_lora_proj:
            ApT = [
                persist.tile([128, R], F16, tag=f"ApT{i}", name=f"ApT{i}")
                for i in range(2)
            ]
            BpT = persist.tile([R, C], F16, tag="BpT", name="BpT")
            LpT = persist.tile([R, T], F16, tag="LpT", name="LpT")

        # ---- DMA weights + x^T (T-block-major so qkT(tb=0) unblocks early) ----
        for j in range(NC_):
            nc.sync.dma_start(
                xT[j][:, 0:512], d["xT"][j * 128 : (j + 1) * 128, 0:512]
            )
            nc.sync.dma_start(wqkT[j][:, :], d["wqkT"][j * 128 : (j + 1) * 128, :])
            nc.sync.dma_start(wvT[j][:, :], d["wvT"][j * 128 : (j + 1) * 128, :])
        for tbk in range(1, 4):
            for j in range(NC_):
                nc.sync.dma_start(
                    xT[j][:, tbk * 512 : (tbk + 1) * 512],
                    d["xT"][j * 128 : (j + 1) * 128, tbk * 512 : (tbk + 1) * 512],
                )
        for i in range(2):
            nc.sync.dma_start(wpT[i][:, :], d["wpT"][i * 128 : (i + 1) * 128, :])
        if use_lora_attn:
            for j in range(NC_):
                nc.sync.dma_start(AatT[j][:, :], d["aatT"][j * 128 : (j + 1) * 128, :])
            nc.sync.dma_start(BqkT[:, :], d["bqkT"][:, :])
            nc.sync.dma_start(BvT[:, :], d["bvT"][:, :])
        if use_lora_proj:
            for i in range(2):
                nc.sync.dma_start(ApT[i][:, :], d["apT"][i * 128 : (i + 1) * 128, :])
            nc.sync.dma_start(BpT[:, :], d["bpT"][:, :])

        def fq_chain(dst_slice, src_ps, w, reshaped=False):
            """fake_quant: dst = (clip(round(src/scale + zp), 0, 255) - zp) * scale

            round-to-nearest-even comes from the fp32 ALU rounding of
            (x + 1.5*2^23) inside the dual-op instruction."""
            t1 = fqp.tile([128, w], F32, tag="fq", name="fq1")
            nc.vector.tensor_scalar(t1[:, :], src_ps, inv_ap, zp_ap, OP.mult, OP.add)
            t2 = fqp.tile([128, w], F32, tag="fq", name="fq2")
            nc.vector.tensor_scalar(t2[:, :], t1[:, :], 0.0, QMAX, OP.max, OP.min)
            t3 = fqp.tile([128, w], F32, tag="fq", name="fq3")
            nc.vector.tensor_scalar(t3[:, :], t2[:, :], MAGIC, MAGIC, OP.add, OP.subtract)
            src = t3[:, :].rearrange("p (h c) -> p h c", c=HD) if reshaped else t3[:, :]
            nc.vector.tensor_scalar(dst_slice, src, zp_ap, None, OP.subtract)

        # ================= pipelined main loop over q-blocks =================
        for tb in range(4):
            # ---- LT block (lora attn intermediate) ----
            if use_lora_attn:
                ps = psB.tile([R, 512], F32, tag="mm", name="lt_ps")
                for j in range(NC_):
                    nc.tensor.matmul(
                        ps[:, :],
                        AatT[j][:, :],
                        xT[j][:, tb * 512 : (tb + 1) * 512],
                        start=(j == 0),
                        stop=(j == NC_ - 1),
                    )
                nc.scalar.mul(LT[:, tb * 512 : (tb + 1) * 512], ps[:, :], ALPHA_OVER_R)

            # ---- qkT block: [q;k]^T channels for this T-block.
            # Two 256-col halves accumulate side by side so each weight tile
            # is loaded once for two matmuls (deduped post-compile).
            for ct in range(4):
                pss = [
                    psB.tile([128, 256], F32, tag="mm", name=f"qk_ps{hb}")
                    for hb in range(2)
                ]
                last = NC_ - 1 if not (use_lora_attn or use_bias) else None
                for j in range(NC_):
                    for hb in range(2):
                        nc.tensor.matmul(
                            pss[hb][:, :],
                            wqkT[j][:, ct * 128 : (ct + 1) * 128],
                            xT[j][:, tb * 512 + hb * 256 : tb * 512 + hb * 256 + 256],
                            start=(j == 0),
                            stop=(j == last),
                        )
                if use_lora_attn:
                    for hb in range(2):
                        nc.tensor.matmul(
                            pss[hb][:, :],
                            BqkT[:, ct * 128 : (ct + 1) * 128],
                            LT[:, tb * 512 + hb * 256 : tb * 512 + hb * 256 + 256],
                            start=False,
                            stop=(not use_bias),
                        )
                if use_bias:
                    for hb in range(2):
                        nc.tensor.matmul(
                            pss[hb][:, :],
                            bqk_row[:, ct * 128 : (ct + 1) * 128],
                            ones_row[:, 0:256],
                            start=False,
                            stop=True,
                        )
                for hb in range(2):
                    dst = qkT[ct][:, tb * 512 + hb * 256 : tb * 512 + hb * 256 + 256]
                    if ct < 2:
                        nc.vector.tensor_copy(dst, pss[hb][:, :])
                    else:
                        fq_chain(dst, pss[hb][:, :], 256)

            # ---- V natural for this block's 4 T-tiles ----
            for t in range(4 * tb, 4 * tb + 4):
                nc.gpsimd.memset(Vaug[t][:, :], 1.0)
                ps = psB.tile([128, CH], F32, tag="mm", name="v_ps")
                last = NC_ - 1 if not (use_lora_attn or use_bias) else None
                for j in range(NC_):
                    nc.tensor.matmul(
                        ps[:, :],
                        xT[j][:, t * 128 : (t + 1) * 128],
                        wvT[j][:, :],
                        start=(j == 0),
                        stop=(j == last),
                    )
                if use_lora_attn:
                    nc.tensor.matmul(
                        ps[:, :],
                        LT[:, t * 128 : (t + 1) * 128],
                        BvT[:, :],
                        start=False,
                        stop=(not use_bias),
                    )
                if use_bias:
                    nc.tensor.matmul(
                        ps[:, :], ones_row[:, 0:128], bv_row[:, :], start=False, stop=True
                    )
                vdst = Vaug[t][:, :].rearrange("p (h c) -> p h c", c=HD + 1)[:, :, 0:HD]
                fq_chain(vdst, ps[:, :], CH, reshaped=True)

            # ---- attention for q-block tb (S^T layout, head pairs) ----
            qb = tb
            for hp in range(2):  # head pair (2hp, 2hp+1)
                qt = qkT[hp]
                kt = qkT[2 + hp]
                qsl0 = qt[0:64, qb * 512 : (qb + 1) * 512]
                qsl1 = qt[64:128, qb * 512 : (qb + 1) * 512]
                nj = 4 * qb + 4
                ex_tiles = []
                for j in range(nj):
                    jl = j - 4 * qb
                    lo = max(jl, 0) * 128  # q-cols < lo are never read for this k-tile
                    ps = psS.tile([128, 1024], F32, tag="st", name="st_ps")
                    # two concurrent K=64 matmuls in PE row-groups 0-63 / 64-127
                    nc.tensor.matmul(
                        ps[:, lo:512],
                        kt[0:64, j * 128 : (j + 1) * 128],
                        qsl0[:, lo:512],
                        start=True,
                        stop=True,
                    )
                    nc.tensor.matmul(
                        ps[:, 512 + lo : 1024],
                        kt[64:128, j * 128 : (j + 1) * 128],
                        qsl1[:, lo:512],
                        start=True,
                        stop=True,
                    )
                    ex = exp_pool.tile([128, 1024], F16, tag="ex", name=f"ex{j}")
                    exv = ex[:, :].rearrange("p (h q) -> p h q", q=512)[:, :, lo:512]
                    psv = ps[:, :].rearrange("p (h q) -> p h q", q=512)[:, :, lo:512]
                    nc.scalar.activation(exv, psv, AF.Exp, scale=es_ap)
                    if jl >= 0:
                        # diagonal k-tile: only the q-slice qi == jl straddles the
                        # causal boundary (qi < jl slices are never read by PV,
                        # qi > jl slices are fully valid) -> one triangle-mask
                        # multiply covering both head halves
                        exd = ex[:, :].rearrange("p (h q) -> p h q", q=512)[
                            :, :, jl * 128 : jl * 128 + 128
                        ]
                        nc.vector.tensor_tensor(
                            exd,
                            exd,
                            maskt[:, :]
                            .rearrange("p (o f) -> p o f", o=1)
                            .broadcast_to([128, 2, 128]),
                            OP.mult,
                        )
                    ex_tiles.append(ex)
                for hh in range(2):
                    h = 2 * hp + hh
                    pvp4 = psV.tile([128, 4 * (HD + 1)], F32, tag="pv", name="pv_ps")
                    for qi in range(4):
                        qig = 4 * qb + qi
                        for j in range(qig + 1):
                            nc.tensor.matmul(
                                pvp4[:, qi * (HD + 1) : (qi + 1) * (HD + 1)],
                                ex_tiles[j][
                                    :, hh * 512 + qi * 128 : hh * 512 + qi * 128 + 128
                                ],
                                Vaug[j][:, h * (HD + 1) : (h + 1) * (HD + 1)],
                                start=(j == 0),
                                stop=(j == qig),
                            )
                    pv4v = pvp4[:, :].rearrange("p (q c) -> p q c", c=HD + 1)
                    rcp4 = rcpp.tile([128, 4], F32, tag="rcp", name="rcp4")
                    nc.vector.reciprocal(rcp4[:, :], pv4v[:, :, HD])
                    rcp4s = rcpp.tile([128, 4], F32, tag="rcp", name="rcp4s")
                    nc.vector.tensor_scalar(
                        rcp4s[:, :], rcp4[:, :], sc_ap, None, OP.mult
                    )
                    dstv = attnN[qb][:, :].rearrange("p (q c) -> p q c", c=CH)[
                        :, :, h * HD : (h + 1) * HD
                    ]
                    nc.vector.tensor_tensor(
                        dstv,
                        pv4v[:, :, 0:HD],
                        rcp4s[:, :]
                        .rearrange("p (q o) -> p q o", o=1)
                        .broadcast_to([128, 4, HD]),
                        OP.mult,
                    )
            # ---- transpose attention output into attnT[cb] [128(ch), T] ----
            for qp in range(2):  # pairs of q-tiles
                tp4 = psV.tile([128, 512], F16, tag="pv", name="tp4")
                for ti in range(2):
                    qi = 2 * qp + ti
                    for cb in range(2):
                        nc.tensor.transpose(
                            tp4[:, (2 * ti + cb) * 128 : (2 * ti + cb) * 128 + 128],
                            attnN[qb][:, qi * CH + cb * 128 : qi * CH + (cb + 1) * 128],
                            id16[:, :],
                        )
                tt0 = 4 * qb + 2 * qp
                dstv = (
                    attnTa[:, :]
                    .rearrange("p (cb t) -> p cb t", cb=2)[
                        :, :, tt0 * 128 : (tt0 + 2) * 128
                    ]
                    .rearrange("p cb (ti f) -> p ti cb f", f=128)
                )
                srcv = tp4[:, :].rearrange("p (ti cb f) -> p ti cb f", ti=2, cb=2)
                nc.vector.tensor_copy(dstv, srcv)
            # ---- LpT block ----
            if use_lora_proj:
                ps = psB.tile([R, 512], F32, tag="mm", name="lp_ps")
                for cb in range(2):
                    nc.tensor.matmul(
                        ps[:, :],
                        ApT[cb][:, :],
                        attnT[cb][:, qb * 512 : (qb + 1) * 512],
                        start=(cb == 0),
                        stop=(cb == 1),
                    )
                nc.scalar.mul(LpT[:, qb * 512 : (qb + 1) * 512], ps[:, :], ALPHA_OVER_R)
            # ---- proj partial for this q-block ----
            for qi in range(4):
                tt = 4 * qb + qi
                po_t = outp.tile([128, C], F16, tag="po", name=f"po{tt}")
                ps2s = [
                    psB.tile([128, 512], F32, tag="mm", name=f"pj_ps{nb}")
                    for nb in range(2)
                ]
                for cb in range(2):
                    for nb in range(2):
                        nc.tensor.matmul(
                            ps2s[nb][:, :],
                            attnT[cb][:, tt * 128 : (tt + 1) * 128],
                            wpT[cb][:, nb * 512 : (nb + 1) * 512],
                            start=(cb == 0),
                            stop=(cb == 1 and not use_lora_proj),
                        )
                if use_lora_proj:
                    for nb in range(2):
                        nc.tensor.matmul(
                            ps2s[nb][:, :],
                            LpT[:, tt * 128 : (tt + 1) * 128],
                            BpT[:, nb * 512 : (nb + 1) * 512],
                            start=False,
                            stop=True,
                        )
                for nb in range(2):
                    nc.vector.tensor_copy(po_t[:, nb * 512 : (nb + 1) * 512], ps2s[nb][:, :])
                nc.sync.dma_start(d["out"][tt * 128 : (tt + 1) * 128, :], po_t[:, :])


def _build_program(use_bias, use_lora_attn, use_lora_proj):
    nc = bacc.Bacc("TRN2", target_bir_lowering=False, debug=False, num_devices=N_CORES)

    def din(name, shape, dt=F16):
        return nc.dram_tensor(name, shape, dt, kind="ExternalInput").ap()

    d = {
        "xT": din("xT", [C, T]),
        "wqkT": din("wqkT", [C, 2 * CH]),
        "wvT": din("wvT", [C, CH]),
        "wpT": din("wpT", [CH, C]),
        "aatT": din("aatT", [C, R]),
        "bqkT": din("bqkT", [R, 2 * CH]),
        "bvT": din("bvT", [R, CH]),
        "apT": din("apT", [CH, R]),
        "bpT": din("bpT", [R, C]),
        "bqk": din("bqk", [1, 2 * CH]),
        "bv": din("bv", [1, CH]),
        "consts": din("consts", [128, 4], F32),
        "id16": din("id16", [128, 128]),
        "masks": din("masks", [128, 128]),
        "out": nc.dram_tensor("out", [T, C], F16, kind="ExternalOutput").ap(),
    }
    with tile.TileContext(nc) as tc:
        _build_body(nc, tc, d, use_bias, use_lora_attn, use_lora_proj)
    nc.compile()
    _dedupe_ldweights(nc)
    return nc


def _ap_key(ap):
    try:
        t = ap.tensor_name if hasattr(ap, "tensor_name") else None
        return (t, str(ap))
    except Exception:
        return (None, object())


def _dedupe_ldweights(nc):
    """Remove back-to-back InstLdweights that reload identical weights.

    Safe only when the duplicate has no semaphore waits/updates (any guarded
    rewrite of the weight region would carry its wait on the ldweights) and
    the intervening instruction is a single non-transpose matmul."""
    removed = 0
    pe = mybir.EngineType.PE
    for blk in nc.m.functions[0].blocks:
        insts = blk.instructions
        keep = []
        prev_key = None
        for inst in insts:
            if getattr(inst, "engine", None) != pe:
                keep.append(inst)
                continue
            t = type(inst).__name__
            if t == "InstLdweights":
                si = inst.sync_info
                clean = si is None or (not si.on_wait and not si.on_update)
                key = str(inst.ins[0])
                if clean and prev_key is not None and key == prev_key:
                    removed += 1
                    continue
                prev_key = key
            elif t == "InstMatmult":
                if getattr(inst, "is_transpose", False):
                    prev_key = None
            keep.append(inst)
        if len(keep) != len(insts):
            blk.instructions = keep
    return removed


_CACHE = {}


def get_program(use_bias=True, use_lora_attn=True, use_lora_proj=True):
    key = (use_bias, use_lora_attn, use_lora_proj)
    if key not in _CACHE:
        _CACHE[key] = _build_program(*key)
    return _CACHE[key]


def make_in_maps(
    hidden_states, W_attn, b_attn, A_attn, B_attn, W_proj, b_proj, A_proj, B_proj,
    kv_scale, kv_zp,
):
    f32, f16 = np.float32, np.float16
    hidden_states = np.asarray(hidden_states, f32)
    W_attn = np.asarray(W_attn, f32)
    b_attn = np.asarray(b_attn, f32)
    A_attn = np.asarray(A_attn, f32)
    B_attn = np.asarray(B_attn, f32)
    W_proj = np.asarray(W_proj, f32)
    A_proj = np.asarray(A_proj, f32)
    B_proj = np.asarray(B_proj, f32)
    scale = f32(np.asarray(kv_scale, f32).reshape(-1)[0])
    zp = f32(np.asarray(kv_zp, f32).reshape(-1)[0])

    consts = np.zeros((128, 4), f32)
    consts[:, 0] = f32(1.0) / scale
    consts[:, 1] = zp
    consts[:, 2] = scale
    consts[:, 3] = np.float32(0.125) * scale

    # id16 + causal masks
    id16 = np.eye(128, dtype=f16)
    iota_p = np.arange(128)[:, None]
    iota_f = np.arange(512)[None, :]
    masks = (iota_f[:, :128] - iota_p >= 0).astype(f16)  # [128,128] lower=0 triangle

    ct = lambda a: np.ascontiguousarray(a).astype(f16)
    xTs = [ct(hidden_states[b].T) for b in range(B)]
    bpT = ct(B_proj.T)

    in_maps = []
    for c in range(N_CORES):
        b = c // 4
        hg = c % 4
        qs = slice(hg * CH, (hg + 1) * CH)
        ks = slice(C + hg * CH, C + (hg + 1) * CH)
        vs = slice(2 * C + hg * CH, 2 * C + (hg + 1) * CH)
        wqk = np.concatenate([W_attn[qs], W_attn[ks]], axis=0)
        bqkl = np.concatenate([B_attn[qs], B_attn[ks]], axis=0)
        in_maps.append(
            {
                "xT": xTs[b],
                "wqkT": ct(wqk.T),
                "wvT": ct(W_attn[vs].T),
                "wpT": ct(W_proj[:, hg * CH : (hg + 1) * CH].T),
                "aatT": ct(A_attn.T),
                "bqkT": ct(bqkl.T),
                "bvT": ct(B_attn[vs].T),
                "apT": ct(A_proj[:, hg * CH : (hg + 1) * CH].T),
                "bpT": bpT,
                "bqk": ct(np.concatenate([b_attn[qs], b_attn[ks]])[None, :]),
                "bv": ct(b_attn[vs][None, :]),
                "consts": consts,
                "id16": id16,
                "masks": masks,
            }
        )
    return in_maps


def variant_flags(b_attn, B_attn, B_proj):
    return (
        bool(np.any(np.asarray(b_attn))),
        bool(np.any(np.asarray(B_attn))),
        bool(np.any(np.asarray(B_proj))),
    )


def assemble_output(results, b_proj):
    out = np.zeros((B, T, C), np.float32)
    for c in range(N_CORES):
        out[c // 4] += results[c]["out"].astype(np.float32)
    out += np.asarray(b_proj, np.float32)[None, None, :]
    return out


def kernel(**inputs):
    flags = variant_flags(inputs["b_attn"], inputs["B_attn"], inputs["B_proj"])
    nc = get_program(*flags)
    in_maps = make_in_maps(**inputs)
    res = run_bass_kernel_spmd(nc, in_maps, core_ids=list(range(N_CORES)))
    return assemble_output(res.results, inputs["b_proj"])


# revision 19
# speedup vs baseline: 1.0056x; 1.0056x over previous
"""CPT attention (QKV+LoRA -> fake-quant KV -> causal attention -> proj+LoRA)
as a Bass/Tile kernel on 8 TRN2 NeuronCores.

Sharding: data parallel over batch (2) x tensor parallel over heads (16/4=4
per core), Megatron-style. Each core computes qkv for its 4 heads from the
full hidden_states[b], runs causal attention locally, and produces a partial
projection output [T, C]; the host sums the 4 tensor-parallel partials per
batch and adds b_proj.

Device-side design notes:
- All matmul operands f16 (1 col/cycle on PE vs 4 for f32), fp32 PSUM accum.
- Host pre-transposes x and all weights to f16 (no on-device load transposes).
- Scores are computed transposed (S^T[k, q]) so the softmax reduction (over k)
  is done by a ones column appended to V: the PV matmul emits the softmax
  denominator as an extra output column. No row-max subtraction needed
  (scores here are O(1)).
- Head pairs share one [128,1024] score PSUM tile: the two K=64 matmuls hit
  partition row-groups 0-63/64-127 and run concurrently in the PE array.
- fake_quant rounding uses the fp32 +/- 1.5*2^23 trick (round-to-nearest-even,
  matching jnp.round).
- Causal masking: exp, then multiply by host-provided 0/1 mask tiles (DVE).
- The whole kernel is pipelined over q-blocks: qkv-projection, V, attention,
  and output-projection for block tb interleave.
- LoRA / bias contributions are compiled out when the corresponding inputs are
  all-zero (separate cached program variants).
"""

import numpy as np

import concourse.bass as bass
import concourse.bacc as bacc
import concourse.mybir as mybir
import concourse.tile as tile
from concourse.bass_utils import run_bass_kernel_spmd

AF = mybir.ActivationFunctionType
OP = mybir.AluOpType

B, T, C = 2, 2048, 1024
H, HD = 16, 64
R = 16
ALPHA_OVER_R = 2.0
QMAX = 255.0
MAGIC = 12582912.0  # 1.5 * 2**23: fp32 add/sub rounds to nearest-even integer
N_CORES = 8
HPC = 4  # heads per core
CH = HPC * HD  # 256 channels (per each of q/k/v) per core
NT = T // 128  # 16 T-tiles
NC_ = C // 128  # 8 C-tiles
F16 = mybir.dt.float16
F32 = mybir.dt.float32


def _build_body(nc, tc, d, use_bias, use_lora_attn, use_lora_proj):
    import contextlib

    ctx = contextlib.ExitStack()
    with ctx:
        persist = ctx.enter_context(tc.tile_pool(name="persist", bufs=1))
        fqp = ctx.enter_context(tc.tile_pool(name="fqp", bufs=6))
        exp_pool = ctx.enter_context(tc.tile_pool(name="exp_pool", bufs=18))
        outp = ctx.enter_context(tc.tile_pool(name="outp", bufs=3))
        rcpp = ctx.enter_context(tc.tile_pool(name="rcpp", bufs=4))
        psS = ctx.enter_context(
            tc.tile_pool(name="psS", bufs=2, space=bass.MemorySpace.PSUM)
        )
        psB = ctx.enter_context(
            tc.tile_pool(name="psB", bufs=2, space=bass.MemorySpace.PSUM)
        )
        psV = ctx.enter_context(
            tc.tile_pool(name="psV", bufs=2, space=bass.MemorySpace.PSUM)
        )

        # ---- constants (all DMA'd; keeps GPSIMD off the startup path) ----
        consts = persist.tile([128, 4], F32, tag="consts", name="consts")
        nc.sync.dma_start(consts[:, :], d["consts"][:, :])
        inv_ap = consts[:, 0:1]
        zp_ap = consts[:, 1:2]
        sc_ap = consts[:, 2:3]
        es_ap = consts[:, 3:4]  # 0.125 * kv_scale (scores use integer-valued K)
        id16 = persist.tile([128, 128], F16, tag="id16", name="id16")
        nc.sync.dma_start(id16[:, :], d["id16"][:, :])
        maskt = persist.tile([128, 128], F16, tag="maskt", name="maskt")
        nc.sync.dma_start(maskt[:, :], d["masks"][:, :])
        if use_bias:
            ones_row = persist.tile([1, 512], F16, tag="ones_row", name="ones_row")
            nc.gpsimd.memset(ones_row[:, :], 1.0)
            bqk_row = persist.tile([1, 2 * CH], F16, tag="bqk_row", name="bqk_row")
            nc.sync.dma_start(bqk_row[:, :], d["bqk"][:, :])
            bv_row = persist.tile([1, CH], F16, tag="bv_row", name="bv_row")
            nc.sync.dma_start(bv_row[:, :], d["bv"][:, :])

        # ---- persistent f16 tensors (DMA'd pre-transposed from host) ----
        xT = [persist.tile([128, T], F16, tag=f"xT{j}", name=f"xT{j}") for j in range(NC_)]
        wqkT = [
            persist.tile([128, 2 * CH], F16, tag=f"wqkT{j}", name=f"wqkT{j}")
            for j in range(NC_)
        ]
        wvT = [
            persist.tile([128, CH], F16, tag=f"wvT{j}", name=f"wvT{j}")
            for j in range(NC_)
        ]
        wpT = [
            persist.tile([128, C], F16, tag=f"wpT{i}", name=f"wpT{i}") for i in range(2)
        ]
        qkT = [
            persist.tile([128, T], F16, tag=f"qkT{i}", name=f"qkT{i}") for i in range(4)
        ]
        Vaug = [
            persist.tile([128, HPC * (HD + 1)], F16, tag=f"Vaug{t}", name=f"Vaug{t}")
            for t in range(NT)
        ]
        attnN = [
            persist.tile([128, 4 * CH], F16, tag=f"attnN{qb}", name=f"attnN{qb}")
            for qb in range(4)
        ]  # [128, qi*256 + ch] per q-block
        attnTa = persist.tile([128, 2 * T], F16, tag="attnTa", name="attnTa")
        attnT = [attnTa[:, cb * T : (cb + 1) * T] for cb in range(2)]
        if use_lora_attn:
            AatT = [
                persist.tile([128, R], F16, tag=f"AatT{j}", name=f"AatT{j}")
                for j in range(NC_)
            ]
            BqkT = persist.tile([R, 2 * CH], F16, tag="BqkT", name="BqkT")
            BvT = persist.tile([R, CH], F16, tag="BvT", name="BvT")
            LT = persist.tile([R, T], F16, tag="LT", name="LT")
        if use_lora_proj:
            ApT = [
                persist.tile([128, R], F16, tag=f"ApT{i}", name=f"ApT{i}")
                for i in range(2)
            ]
            BpT = persist.tile([R, C], F16, tag="BpT", name="BpT")
            LpT = persist.tile([R, T], F16, tag="LpT", name="LpT")

        # ---- DMA weights + x^T (T-block-major so qkT(tb=0) unblocks early) ----
        for j in range(NC_):
            nc.sync.dma_start(
                xT[j][:, 0:512], d["xT"][j * 128 : (j + 1) * 128, 0:512]
            )
            nc.sync.dma_start(wqkT[j][:, :], d["wqkT"][j * 128 : (j + 1) * 128, :])
            nc.sync.dma_start(wvT[j][:, :], d["wvT"][j * 128 : (j + 1) * 128, :])
        for tbk in range(1, 4):
            for j in range(NC_):
                nc.sync.dma_start(
                    xT[j][:, tbk * 512 : (tbk + 1) * 512],
                    d["xT"][j * 128 : (j + 1) * 128, tbk * 512 : (tbk + 1) * 512],
                )
        for i in range(2):
            nc.sync.dma_start(wpT[i][:, :], d["wpT"][i * 128 : (i + 1) * 128, :])
        if use_lora_attn:
            for j in range(NC_):
                nc.sync.dma_start(AatT[j][:, :], d["aatT"][j * 128 : (j + 1) * 128, :])
            nc.sync.dma_start(BqkT[:, :], d["bqkT"][:, :])
            nc.sync.dma_start(BvT[:, :], d["bvT"][:, :])
        if use_lora_proj:
            for i in range(2):
                nc.sync.dma_start(ApT[i][:, :], d["apT"][i * 128 : (i + 1) * 128, :])
            nc.sync.dma_start(BpT[:, :], d["bpT"][:, :])

        def fq_chain(dst_slice, src_ps, w, reshaped=False):
            """fake_quant: dst = (clip(round(src/scale + zp), 0, 255) - zp) * scale

            round-to-nearest-even comes from the fp32 ALU rounding of
            (x + 1.5*2^23) inside the dual-op instruction."""
            t1 = fqp.tile([128, w], F32, tag="fq", name="fq1")
            nc.vector.tensor_scalar(t1[:, :], src_ps, inv_ap, zp_ap, OP.mult, OP.add)
            t2 = fqp.tile([128, w], F32, tag="fq", name="fq2")
            nc.vector.tensor_scalar(t2[:, :], t1[:, :], 0.0, QMAX, OP.max, OP.min)
            t3 = fqp.tile([128, w], F32, tag="fq", name="fq3")
            nc.vector.tensor_scalar(t3[:, :], t2[:, :], MAGIC, MAGIC, OP.add, OP.subtract)
            src = t3[:, :].rearrange("p (h c) -> p h c", c=HD) if reshaped else t3[:, :]
            nc.vector.tensor_scalar(dst_slice, src, zp_ap, None, OP.subtract)

        # ================= pipelined main loop over q-blocks =================
        for tb in range(4):
            # ---- LT block (lora attn intermediate) ----
            if use_lora_attn:
                ps = psB.tile([R, 512], F32, tag="mm", name="lt_ps")
                for j in range(NC_):
                    nc.tensor.matmul(
                        ps[:, :],
                        AatT[j][:, :],
                        xT[j][:, tb * 512 : (tb + 1) * 512],
                        start=(j == 0),
                        stop=(j == NC_ - 1),
                    )
                nc.scalar.mul(LT[:, tb * 512 : (tb + 1) * 512], ps[:, :], ALPHA_OVER_R)

            # ---- qkT block: [q;k]^T channels for this T-block.
            # Two 256-col halves accumulate side by side so each weight tile
            # is loaded once for two matmuls (deduped post-compile).
            for ct in range(4):
                pss = [
                    psB.tile([128, 256], F32, tag="mm", name=f"qk_ps{hb}")
                    for hb in range(2)
                ]
                last = NC_ - 1 if not (use_lora_attn or use_bias) else None
                for j in range(NC_):
                    for hb in range(2):
                        nc.tensor.matmul(
                            pss[hb][:, :],
                            wqkT[j][:, ct * 128 : (ct + 1) * 128],
                            xT[j][:, tb * 512 + hb * 256 : tb * 512 + hb * 256 + 256],
                            start=(j == 0),
                            stop=(j == last),
                        )
                if use_lora_attn:
                    for hb in range(2):
                        nc.tensor.matmul(
                            pss[hb][:, :],
                            BqkT[:, ct * 128 : (ct + 1) * 128],
                            LT[:, tb * 512 + hb * 256 : tb * 512 + hb * 256 + 256],
                            start=False,
                            stop=(not use_bias),
                        )
                if use_bias:
                    for hb in range(2):
                        nc.tensor.matmul(
                            pss[hb][:, :],
                            bqk_row[:, ct * 128 : (ct + 1) * 128],
                            ones_row[:, 0:256],
                            start=False,
                            stop=True,
                        )
                if ct < 2:
                    for hb in range(2):
                        dst = qkT[ct][:, tb * 512 + hb * 256 : tb * 512 + hb * 256 + 256]
                        nc.vector.tensor_copy(dst, pss[hb][:, :])
                else:
                    t1 = fqp.tile([128, 512], F32, tag="fq", name="kfq1")
                    for hb in range(2):
                        nc.vector.tensor_scalar(
                            t1[:, hb * 256 : (hb + 1) * 256], pss[hb][:, :],
                            inv_ap, zp_ap, OP.mult, OP.add,
                        )
                    t2 = fqp.tile([128, 512], F32, tag="fq", name="kfq2")
                    nc.vector.tensor_scalar(t2[:, :], t1[:, :], 0.0, QMAX, OP.max, OP.min)
                    t3 = fqp.tile([128, 512], F32, tag="fq", name="kfq3")
                    nc.vector.tensor_scalar(
                        t3[:, :], t2[:, :], MAGIC, MAGIC, OP.add, OP.subtract
                    )
                    nc.vector.tensor_scalar(
                        qkT[ct][:, tb * 512 : (tb + 1) * 512], t3[:, :],
                        zp_ap, None, OP.subtract,
                    )

            # ---- V natural for this block's 4 T-tiles (fq batched in pairs) ----
            for tp_ in range(2):
                ts_pair = [4 * tb + 2 * tp_, 4 * tb + 2 * tp_ + 1]
                pss2 = []
                for t in ts_pair:
                    nc.gpsimd.memset(Vaug[t][:, :], 1.0)
                    ps = psB.tile([128, CH], F32, tag="mm", name="v_ps")
                    pss2.append(ps)
                    last = NC_ - 1 if not (use_lora_attn or use_bias) else None
                    for j in range(NC_):
                        nc.tensor.matmul(
                            ps[:, :],
                            xT[j][:, t * 128 : (t + 1) * 128],
                            wvT[j][:, :],
                            start=(j == 0),
                            stop=(j == last),
                        )
                    if use_lora_attn:
                        nc.tensor.matmul(
                            ps[:, :],
                            LT[:, t * 128 : (t + 1) * 128],
                            BvT[:, :],
                            start=False,
                            stop=(not use_bias),
                        )
                    if use_bias:
                        nc.tensor.matmul(
                            ps[:, :], ones_row[:, 0:128], bv_row[:, :],
                            start=False, stop=True,
                        )
                t1 = fqp.tile([128, 512], F32, tag="fq", name="vfq1")
                for i2 in range(2):
                    nc.vector.tensor_scalar(
                        t1[:, i2 * CH : (i2 + 1) * CH], pss2[i2][:, :],
                        inv_ap, zp_ap, OP.mult, OP.add,
                    )
                t2 = fqp.tile([128, 512], F32, tag="fq", name="vfq2")
                nc.vector.tensor_scalar(t2[:, :], t1[:, :], 0.0, QMAX, OP.max, OP.min)
                t3 = fqp.tile([128, 512], F32, tag="fq", name="vfq3")
                nc.vector.tensor_scalar(
                    t3[:, :], t2[:, :], MAGIC, MAGIC, OP.add, OP.subtract
                )
                for i2 in range(2):
                    t = ts_pair[i2]
                    vdst = Vaug[t][:, :].rearrange("p (h c) -> p h c", c=HD + 1)[
                        :, :, 0:HD
                    ]
                    nc.vector.tensor_scalar(
                        vdst,
                        t3[:, i2 * CH : (i2 + 1) * CH].rearrange(
                            "p (h c) -> p h c", c=HD
                        ),
                        zp_ap, None, OP.subtract,
                    )

            # ---- attention for q-block tb (S^T layout, head pairs) ----
            qb = tb
            for hp in range(2):  # head pair (2hp, 2hp+1)
                qt = qkT[hp]
                kt = qkT[2 + hp]
                qsl0 = qt[0:64, qb * 512 : (qb + 1) * 512]
                qsl1 = qt[64:128, qb * 512 : (qb + 1) * 512]
                nj = 4 * qb + 4
                ex_tiles = []
                for j in range(nj):
                    jl = j - 4 * qb
                    lo = max(jl, 0) * 128  # q-cols < lo are never read for this k-tile
                    ps = psS.tile([128, 1024], F32, tag="st", name="st_ps")
                    # two concurrent K=64 matmuls in PE row-groups 0-63 / 64-127
                    nc.tensor.matmul(
                        ps[:, lo:512],
                        kt[0:64, j * 128 : (j + 1) * 128],
                        qsl0[:, lo:512],
                        start=True,
                        stop=True,
                    )
                    nc.tensor.matmul(
                        ps[:, 512 + lo : 1024],
                        kt[64:128, j * 128 : (j + 1) * 128],
                        qsl1[:, lo:512],
                        start=True,
                        stop=True,
                    )
                    ex = exp_pool.tile([128, 1024], F16, tag="ex", name=f"ex{j}")
                    exv = ex[:, :].rearrange("p (h q) -> p h q", q=512)[:, :, lo:512]
                    psv = ps[:, :].rearrange("p (h q) -> p h q", q=512)[:, :, lo:512]
                    nc.scalar.activation(exv, psv, AF.Exp, scale=es_ap)
                    if jl >= 0:
                        # diagonal k-tile: only the q-slice qi == jl straddles the
                        # causal boundary (qi < jl slices are never read by PV,
                        # qi > jl slices are fully valid) -> one triangle-mask
                        # multiply covering both head halves
                        exd = ex[:, :].rearrange("p (h q) -> p h q", q=512)[
                            :, :, jl * 128 : jl * 128 + 128
                        ]
                        nc.vector.tensor_tensor(
                            exd,
                            exd,
                            maskt[:, :]
                            .rearrange("p (o f) -> p o f", o=1)
                            .broadcast_to([128, 2, 128]),
                            OP.mult,
                        )
                    ex_tiles.append(ex)
                for hh in range(2):
                    h = 2 * hp + hh
                    pvp4 = psV.tile([128, 4 * (HD + 1)], F32, tag="pv", name="pv_ps")
                    for qi in range(4):
                        qig = 4 * qb + qi
                        for j in range(qig + 1):
                            nc.tensor.matmul(
                                pvp4[:, qi * (HD + 1) : (qi + 1) * (HD + 1)],
                                ex_tiles[j][
                                    :, hh * 512 + qi * 128 : hh * 512 + qi * 128 + 128
                                ],
                                Vaug[j][:, h * (HD + 1) : (h + 1) * (HD + 1)],
                                start=(j == 0),
                                stop=(j == qig),
                            )
                    pv4v = pvp4[:, :].rearrange("p (q c) -> p q c", c=HD + 1)
                    rcp4 = rcpp.tile([128, 4], F32, tag="rcp", name="rcp4")
                    nc.vector.reciprocal(rcp4[:, :], pv4v[:, :, HD])
                    rcp4s = rcpp.tile([128, 4], F32, tag="rcp", name="rcp4s")
                    nc.vector.tensor_scalar(
                        rcp4s[:, :], rcp4[:, :], sc_ap, None, OP.mult
                    )
                    dstv = attnN[qb][:, :].rearrange("p (q c) -> p q c", c=CH)[
                        :, :, h * HD : (h + 1) * HD
                    ]
                    nc.vector.tensor_tensor(
                        dstv,
                        pv4v[:, :, 0:HD],
                        rcp4s[:, :]
                        .rearrange("p (q o) -> p q o", o=1)
                        .broadcast_to([128, 4, HD]),
                        OP.mult,
                    )
            # ---- transpose attention output into attnT[cb] [128(ch), T] ----
            for qp in range(2):  # pairs of q-tiles
                tp4 = psV.tile([128, 512], F16, tag="pv", name="tp4")
                for ti in range(2):
                    qi = 2 * qp + ti
                    for cb in range(2):
                        nc.tensor.transpose(
                            tp4[:, (2 * ti + cb) * 128 : (2 * ti + cb) * 128 + 128],
                            attnN[qb][:, qi * CH + cb * 128 : qi * CH + (cb + 1) * 128],
                            id16[:, :],
                        )
                tt0 = 4 * qb + 2 * qp
                dstv = (
                    attnTa[:, :]
                    .rearrange("p (cb t) -> p cb t", cb=2)[
                        :, :, tt0 * 128 : (tt0 + 2) * 128
                    ]
                    .rearrange("p cb (ti f) -> p ti cb f", f=128)
                )
                srcv = tp4[:, :].rearrange("p (ti cb f) -> p ti cb f", ti=2, cb=2)
                nc.vector.tensor_copy(dstv, srcv)
            # ---- LpT block ----
            if use_lora_proj:
                ps = psB.tile([R, 512], F32, tag="mm", name="lp_ps")
                for cb in range(2):
                    nc.tensor.matmul(
                        ps[:, :],
                        ApT[cb][:, :],
                        attnT[cb][:, qb * 512 : (qb + 1) * 512],
                        start=(cb == 0),
                        stop=(cb == 1),
                    )
                nc.scalar.mul(LpT[:, qb * 512 : (qb + 1) * 512], ps[:, :], ALPHA_OVER_R)
            # ---- proj partial for this q-block ----
            for qi in range(4):
                tt = 4 * qb + qi
                po_t = outp.tile([128, C], F16, tag="po", name=f"po{tt}")
                ps2s = [
                    psB.tile([128, 512], F32, tag="mm", name=f"pj_ps{nb}")
                    for nb in range(2)
                ]
                for cb in range(2):
                    for nb in range(2):
                        nc.tensor.matmul(
                            ps2s[nb][:, :],
                            attnT[cb][:, tt * 128 : (tt + 1) * 128],
                            wpT[cb][:, nb * 512 : (nb + 1) * 512],
                            start=(cb == 0),
                            stop=(cb == 1 and not use_lora_proj),
                        )
                if use_lora_proj:
                    for nb in range(2):
                        nc.tensor.matmul(
                            ps2s[nb][:, :],
                            LpT[:, tt * 128 : (tt + 1) * 128],
                            BpT[:, nb * 512 : (nb + 1) * 512],
                            start=False,
                            stop=True,
                        )
                for nb in range(2):
                    nc.vector.tensor_copy(po_t[:, nb * 512 : (nb + 1) * 512], ps2s[nb][:, :])
                nc.sync.dma_start(d["out"][tt * 128 : (tt + 1) * 128, :], po_t[:, :])


def _build_program(use_bias, use_lora_attn, use_lora_proj):
    nc = bacc.Bacc("TRN2", target_bir_lowering=False, debug=False, num_devices=N_CORES)

    def din(name, shape, dt=F16):
        return nc.dram_tensor(name, shape, dt, kind="ExternalInput").ap()

    d = {
        "xT": din("xT", [C, T]),
        "wqkT": din("wqkT", [C, 2 * CH]),
        "wvT": din("wvT", [C, CH]),
        "wpT": din("wpT", [CH, C]),
        "aatT": din("aatT", [C, R]),
        "bqkT": din("bqkT", [R, 2 * CH]),
        "bvT": din("bvT", [R, CH]),
        "apT": din("apT", [CH, R]),
        "bpT": din("bpT", [R, C]),
        "bqk": din("bqk", [1, 2 * CH]),
        "bv": din("bv", [1, CH]),
        "consts": din("consts", [128, 4], F32),
        "id16": din("id16", [128, 128]),
        "masks": din("masks", [128, 128]),
        "out": nc.dram_tensor("out", [T, C], F16, kind="ExternalOutput").ap(),
    }
    with tile.TileContext(nc) as tc:
        _build_body(nc, tc, d, use_bias, use_lora_attn, use_lora_proj)
    nc.compile()
    _dedupe_ldweights(nc)
    return nc


def _ap_key(ap):
    try:
        t = ap.tensor_name if hasattr(ap, "tensor_name") else None
        return (t, str(ap))
    except Exception:
        return (None, object())


def _dedupe_ldweights(nc):
    """Remove back-to-back InstLdweights that reload identical weights.

    Safe only when the duplicate has no semaphore waits/updates (any guarded
    rewrite of the weight region would carry its wait on the ldweights) and
    the intervening instruction is a single non-transpose matmul."""
    removed = 0
    pe = mybir.EngineType.PE
    for blk in nc.m.functions[0].blocks:
        insts = blk.instructions
        keep = []
        prev_key = None
        for inst in insts:
            if getattr(inst, "engine", None) != pe:
                keep.append(inst)
                continue
            t = type(inst).__name__
            if t == "InstLdweights":
                si = inst.sync_info
                clean = si is None or (not si.on_wait and not si.on_update)
                key = str(inst.ins[0])
                if clean and prev_key is not None and key == prev_key:
                    removed += 1
                    continue
                prev_key = key
            elif t == "InstMatmult":
                if getattr(inst, "is_transpose", False):
                    prev_key = None
            keep.append(inst)
        if len(keep) != len(insts):
            blk.instructions = keep
    return removed


_CACHE = {}


def get_program(use_bias=True, use_lora_attn=True, use_lora_proj=True):
    key = (use_bias, use_lora_attn, use_lora_proj)
    if key not in _CACHE:
        _CACHE[key] = _build_program(*key)
    return _CACHE[key]


def make_in_maps(
    hidden_states, W_attn, b_attn, A_attn, B_attn, W_proj, b_proj, A_proj, B_proj,
    kv_scale, kv_zp,
):
    f32, f16 = np.float32, np.float16
    hidden_states = np.asarray(hidden_states, f32)
    W_attn = np.asarray(W_attn, f32)
    b_attn = np.asarray(b_attn, f32)
    A_attn = np.asarray(A_attn, f32)
    B_attn = np.asarray(B_attn, f32)
    W_proj = np.asarray(W_proj, f32)
    A_proj = np.asarray(A_proj, f32)
    B_proj = np.asarray(B_proj, f32)
    scale = f32(np.asarray(kv_scale, f32).reshape(-1)[0])
    zp = f32(np.asarray(kv_zp, f32).reshape(-1)[0])

    consts = np.zeros((128, 4), f32)
    consts[:, 0] = f32(1.0) / scale
    consts[:, 1] = zp
    consts[:, 2] = scale
    consts[:, 3] = np.float32(0.125) * scale

    # id16 + causal masks
    id16 = np.eye(128, dtype=f16)
    iota_p = np.arange(128)[:, None]
    iota_f = np.arange(512)[None, :]
    masks = (iota_f[:, :128] - iota_p >= 0).astype(f16)  # [128,128] lower=0 triangle

    ct = lambda a: np.ascontiguousarray(a).astype(f16)
    xTs = [ct(hidden_states[b].T) for b in range(B)]
    bpT = ct(B_proj.T)

    in_maps = []
    for c in range(N_CORES):
        b = c // 4
        hg = c % 4
        qs = slice(hg * CH, (hg + 1) * CH)
        ks = slice(C + hg * CH, C + (hg + 1) * CH)
        vs = slice(2 * C + hg * CH, 2 * C + (hg + 1) * CH)
        wqk = np.concatenate([W_attn[qs], W_attn[ks]], axis=0)
        bqkl = np.concatenate([B_attn[qs], B_attn[ks]], axis=0)
        in_maps.append(
            {
                "xT": xTs[b],
                "wqkT": ct(wqk.T),
                "wvT": ct(W_attn[vs].T),
                "wpT": ct(W_proj[:, hg * CH : (hg + 1) * CH].T),
                "aatT": ct(A_attn.T),
                "bqkT": ct(bqkl.T),
                "bvT": ct(B_attn[vs].T),
                "apT": ct(A_proj[:, hg * CH : (hg + 1) * CH].T),
                "bpT": bpT,
                "bqk": ct(np.concatenate([b_attn[qs], b_attn[ks]])[None, :]),
                "bv": ct(b_attn[vs][None, :]),
                "consts": consts,
                "id16": id16,
                "masks": masks,
            }
        )
    return in_maps


def variant_flags(b_attn, B_attn, B_proj):
    return (
        bool(np.any(np.asarray(b_attn))),
        bool(np.any(np.asarray(B_attn))),
        bool(np.any(np.asarray(B_proj))),
    )


def assemble_output(results, b_proj):
    out = np.zeros((B, T, C), np.float32)
    for c in range(N_CORES):
        out[c // 4] += results[c]["out"].astype(np.float32)
    out += np.asarray(b_proj, np.float32)[None, None, :]
    return out


def kernel(**inputs):
    flags = variant_flags(inputs["b_attn"], inputs["B_attn"], inputs["B_proj"])
    nc = get_program(*flags)
    in_maps = make_in_maps(**inputs)
    res = run_bass_kernel_spmd(nc, in_maps, core_ids=list(range(N_CORES)))
    return assemble_output(res.results, inputs["b_proj"])


# revision 20
# speedup vs baseline: 1.0266x; 1.0208x over previous
"""CPT attention (QKV+LoRA -> fake-quant KV -> causal attention -> proj+LoRA)
as a Bass/Tile kernel on 8 TRN2 NeuronCores.

Sharding: data parallel over batch (2) x tensor parallel over heads (16/4=4
per core), Megatron-style. Each core computes qkv for its 4 heads from the
full hidden_states[b], runs causal attention locally, and produces a partial
projection output [T, C]; the host sums the 4 tensor-parallel partials per
batch and adds b_proj.

Device-side design notes:
- All matmul operands f16 (1 col/cycle on PE vs 4 for f32), fp32 PSUM accum.
- Host pre-transposes x and all weights to f16 (no on-device load transposes).
- Scores are computed transposed (S^T[k, q]) so the softmax reduction (over k)
  is done by a ones column appended to V: the PV matmul emits the softmax
  denominator as an extra output column. No row-max subtraction needed
  (scores here are O(1)).
- Head pairs share one [128,1024] score PSUM tile: the two K=64 matmuls hit
  partition row-groups 0-63/64-127 and run concurrently in the PE array.
- fake_quant rounding uses the fp32 +/- 1.5*2^23 trick (round-to-nearest-even,
  matching jnp.round).
- Causal masking: exp, then multiply by host-provided 0/1 mask tiles (DVE).
- The whole kernel is pipelined over q-blocks: qkv-projection, V, attention,
  and output-projection for block tb interleave.
- LoRA / bias contributions are compiled out when the corresponding inputs are
  all-zero (separate cached program variants).
"""

import numpy as np

import concourse.bass as bass
import concourse.bacc as bacc
import concourse.mybir as mybir
import concourse.tile as tile
from concourse.bass_utils import run_bass_kernel_spmd

AF = mybir.ActivationFunctionType
OP = mybir.AluOpType

B, T, C = 2, 2048, 1024
H, HD = 16, 64
R = 16
ALPHA_OVER_R = 2.0
QMAX = 255.0
MAGIC = 12582912.0  # 1.5 * 2**23: fp32 add/sub rounds to nearest-even integer
N_CORES = 8
HPC = 4  # heads per core
CH = HPC * HD  # 256 channels (per each of q/k/v) per core
NT = T // 128  # 16 T-tiles
NC_ = C // 128  # 8 C-tiles
F16 = mybir.dt.float16
F32 = mybir.dt.float32


def _build_body(nc, tc, d, use_bias, use_lora_attn, use_lora_proj):
    import contextlib

    ctx = contextlib.ExitStack()
    with ctx:
        persist = ctx.enter_context(tc.tile_pool(name="persist", bufs=1))
        fqp = ctx.enter_context(tc.tile_pool(name="fqp", bufs=6))
        exp_pool = ctx.enter_context(tc.tile_pool(name="exp_pool", bufs=18))
        outp = ctx.enter_context(tc.tile_pool(name="outp", bufs=3))
        rcpp = ctx.enter_context(tc.tile_pool(name="rcpp", bufs=4))
        psS = ctx.enter_context(
            tc.tile_pool(name="psS", bufs=2, space=bass.MemorySpace.PSUM)
        )
        psB = ctx.enter_context(
            tc.tile_pool(name="psB", bufs=2, space=bass.MemorySpace.PSUM)
        )
        psV = ctx.enter_context(
            tc.tile_pool(name="psV", bufs=2, space=bass.MemorySpace.PSUM)
        )

        # ---- constants (all DMA'd; keeps GPSIMD off the startup path) ----
        consts = persist.tile([128, 4], F32, tag="consts", name="consts")
        nc.sync.dma_start(consts[:, :], d["consts"][:, :])
        inv_ap = consts[:, 0:1]
        zp_ap = consts[:, 1:2]
        sc_ap = consts[:, 2:3]
        es_ap = consts[:, 3:4]  # 0.125 * kv_scale (scores use integer-valued K)
        id16 = persist.tile([128, 128], F16, tag="id16", name="id16")
        nc.sync.dma_start(id16[:, :], d["id16"][:, :])
        maskt = persist.tile([128, 128], F16, tag="maskt", name="maskt")
        nc.sync.dma_start(maskt[:, :], d["masks"][:, :])
        if use_bias:
            ones_row = persist.tile([1, 512], F16, tag="ones_row", name="ones_row")
            nc.gpsimd.memset(ones_row[:, :], 1.0)
            bqk_row = persist.tile([1, 2 * CH], F16, tag="bqk_row", name="bqk_row")
            nc.sync.dma_start(bqk_row[:, :], d["bqk"][:, :])
            bv_row = persist.tile([1, CH], F16, tag="bv_row", name="bv_row")
            nc.sync.dma_start(bv_row[:, :], d["bv"][:, :])

        # ---- persistent f16 tensors (DMA'd pre-transposed from host) ----
        xT = [persist.tile([128, T], F16, tag=f"xT{j}", name=f"xT{j}") for j in range(NC_)]
        wqkT = [
            persist.tile([128, 2 * CH], F16, tag=f"wqkT{j}", name=f"wqkT{j}")
            for j in range(NC_)
        ]
        wvT = [
            persist.tile([128, CH], F16, tag=f"wvT{j}", name=f"wvT{j}")
            for j in range(NC_)
        ]
        wpT = [
            persist.tile([128, C], F16, tag=f"wpT{i}", name=f"wpT{i}") for i in range(2)
        ]
        qkT = [
            persist.tile([128, T], F16, tag=f"qkT{i}", name=f"qkT{i}") for i in range(4)
        ]
        Vaug = [
            persist.tile([128, HPC * (HD + 1)], F16, tag=f"Vaug{t}", name=f"Vaug{t}")
            for t in range(NT)
        ]
        attnN = [
            persist.tile([128, 4 * CH], F16, tag=f"attnN{qb}", name=f"attnN{qb}")
            for qb in range(4)
        ]  # [128, qi*256 + ch] per q-block
        attnTa = persist.tile([128, 2 * T], F16, tag="attnTa", name="attnTa")
        attnT = [attnTa[:, cb * T : (cb + 1) * T] for cb in range(2)]
        if use_lora_attn:
            AatT = [
                persist.tile([128, R], F16, tag=f"AatT{j}", name=f"AatT{j}")
                for j in range(NC_)
            ]
            BqkT = persist.tile([R, 2 * CH], F16, tag="BqkT", name="BqkT")
            BvT = persist.tile([R, CH], F16, tag="BvT", name="BvT")
            LT = persist.tile([R, T], F16, tag="LT", name="LT")
        if use_lora_proj:
            ApT = [
                persist.tile([128, R], F16, tag=f"ApT{i}", name=f"ApT{i}")
                for i in range(2)
            ]
            BpT = persist.tile([R, C], F16, tag="BpT", name="BpT")
            LpT = persist.tile([R, T], F16, tag="LpT", name="LpT")

        # ---- DMA weights + x^T (T-block-major so qkT(tb=0) unblocks early) ----
        for j in range(NC_):
            nc.sync.dma_start(
                xT[j][:, 0:512], d["xT"][j * 128 : (j + 1) * 128, 0:512]
            )
            nc.sync.dma_start(wqkT[j][:, :], d["wqkT"][j * 128 : (j + 1) * 128, :])
        for j in range(NC_):
            nc.sync.dma_start(wvT[j][:, :], d["wvT"][j * 128 : (j + 1) * 128, :])
        for tbk in range(1, 4):
            for j in range(NC_):
                nc.sync.dma_start(
                    xT[j][:, tbk * 512 : (tbk + 1) * 512],
                    d["xT"][j * 128 : (j + 1) * 128, tbk * 512 : (tbk + 1) * 512],
                )
        for i in range(2):
            nc.sync.dma_start(wpT[i][:, :], d["wpT"][i * 128 : (i + 1) * 128, :])
        if use_lora_attn:
            for j in range(NC_):
                nc.sync.dma_start(AatT[j][:, :], d["aatT"][j * 128 : (j + 1) * 128, :])
            nc.sync.dma_start(BqkT[:, :], d["bqkT"][:, :])
            nc.sync.dma_start(BvT[:, :], d["bvT"][:, :])
        if use_lora_proj:
            for i in range(2):
                nc.sync.dma_start(ApT[i][:, :], d["apT"][i * 128 : (i + 1) * 128, :])
            nc.sync.dma_start(BpT[:, :], d["bpT"][:, :])

        def fq_chain(dst_slice, src_ps, w, reshaped=False):
            """fake_quant: dst = (clip(round(src/scale + zp), 0, 255) - zp) * scale

            round-to-nearest-even comes from the fp32 ALU rounding of
            (x + 1.5*2^23) inside the dual-op instruction."""
            t1 = fqp.tile([128, w], F32, tag="fq", name="fq1")
            nc.vector.tensor_scalar(t1[:, :], src_ps, inv_ap, zp_ap, OP.mult, OP.add)
            t2 = fqp.tile([128, w], F32, tag="fq", name="fq2")
            nc.vector.tensor_scalar(t2[:, :], t1[:, :], 0.0, QMAX, OP.max, OP.min)
            t3 = fqp.tile([128, w], F32, tag="fq", name="fq3")
            nc.vector.tensor_scalar(t3[:, :], t2[:, :], MAGIC, MAGIC, OP.add, OP.subtract)
            src = t3[:, :].rearrange("p (h c) -> p h c", c=HD) if reshaped else t3[:, :]
            nc.vector.tensor_scalar(dst_slice, src, zp_ap, None, OP.subtract)

        # ================= pipelined main loop over q-blocks =================
        for tb in range(4):
            # ---- LT block (lora attn intermediate) ----
            if use_lora_attn:
                ps = psB.tile([R, 512], F32, tag="mm", name="lt_ps")
                for j in range(NC_):
                    nc.tensor.matmul(
                        ps[:, :],
                        AatT[j][:, :],
                        xT[j][:, tb * 512 : (tb + 1) * 512],
                        start=(j == 0),
                        stop=(j == NC_ - 1),
                    )
                nc.scalar.mul(LT[:, tb * 512 : (tb + 1) * 512], ps[:, :], ALPHA_OVER_R)

            # ---- qkT block: [q;k]^T channels for this T-block.
            # Two 256-col halves accumulate side by side so each weight tile
            # is loaded once for two matmuls (deduped post-compile).
            for ct in range(4):
                pss = [
                    psB.tile([128, 256], F32, tag="mm", name=f"qk_ps{hb}")
                    for hb in range(2)
                ]
                last = NC_ - 1 if not (use_lora_attn or use_bias) else None
                for j in range(NC_):
                    for hb in range(2):
                        nc.tensor.matmul(
                            pss[hb][:, :],
                            wqkT[j][:, ct * 128 : (ct + 1) * 128],
                            xT[j][:, tb * 512 + hb * 256 : tb * 512 + hb * 256 + 256],
                            start=(j == 0),
                            stop=(j == last),
                        )
                if use_lora_attn:
                    for hb in range(2):
                        nc.tensor.matmul(
                            pss[hb][:, :],
                            BqkT[:, ct * 128 : (ct + 1) * 128],
                            LT[:, tb * 512 + hb * 256 : tb * 512 + hb * 256 + 256],
                            start=False,
                            stop=(not use_bias),
                        )
                if use_bias:
                    for hb in range(2):
                        nc.tensor.matmul(
                            pss[hb][:, :],
                            bqk_row[:, ct * 128 : (ct + 1) * 128],
                            ones_row[:, 0:256],
                            start=False,
                            stop=True,
                        )
                for hb in range(2):
                    dst = qkT[ct][:, tb * 512 + hb * 256 : tb * 512 + hb * 256 + 256]
                    if ct < 2:
                        nc.vector.tensor_copy(dst, pss[hb][:, :])
                    else:
                        fq_chain(dst, pss[hb][:, :], 256)

            # ---- V natural for this block's 4 T-tiles ----
            for t in range(4 * tb, 4 * tb + 4):
                nc.gpsimd.memset(Vaug[t][:, :], 1.0)
                ps = psB.tile([128, CH], F32, tag="mm", name="v_ps")
                last = NC_ - 1 if not (use_lora_attn or use_bias) else None
                for j in range(NC_):
                    nc.tensor.matmul(
                        ps[:, :],
                        xT[j][:, t * 128 : (t + 1) * 128],
                        wvT[j][:, :],
                        start=(j == 0),
                        stop=(j == last),
                    )
                if use_lora_attn:
                    nc.tensor.matmul(
                        ps[:, :],
                        LT[:, t * 128 : (t + 1) * 128],
                        BvT[:, :],
                        start=False,
                        stop=(not use_bias),
                    )
                if use_bias:
                    nc.tensor.matmul(
                        ps[:, :], ones_row[:, 0:128], bv_row[:, :], start=False, stop=True
                    )
                vdst = Vaug[t][:, :].rearrange("p (h c) -> p h c", c=HD + 1)[:, :, 0:HD]
                fq_chain(vdst, ps[:, :], CH, reshaped=True)

            # ---- attention for q-block tb (S^T layout, head pairs) ----
            qb = tb
            for hp in range(2):  # head pair (2hp, 2hp+1)
                qt = qkT[hp]
                kt = qkT[2 + hp]
                qsl0 = qt[0:64, qb * 512 : (qb + 1) * 512]
                qsl1 = qt[64:128, qb * 512 : (qb + 1) * 512]
                nj = 4 * qb + 4
                ex_tiles = []
                for j in range(nj):
                    jl = j - 4 * qb
                    lo = max(jl, 0) * 128  # q-cols < lo are never read for this k-tile
                    ps = psS.tile([128, 1024], F32, tag="st", name="st_ps")
                    # two concurrent K=64 matmuls in PE row-groups 0-63 / 64-127
                    nc.tensor.matmul(
                        ps[:, lo:512],
                        kt[0:64, j * 128 : (j + 1) * 128],
                        qsl0[:, lo:512],
                        start=True,
                        stop=True,
                    )
                    nc.tensor.matmul(
                        ps[:, 512 + lo : 1024],
                        kt[64:128, j * 128 : (j + 1) * 128],
                        qsl1[:, lo:512],
                        start=True,
                        stop=True,
                    )
                    ex = exp_pool.tile([128, 1024], F16, tag="ex", name=f"ex{j}")
                    exv = ex[:, :].rearrange("p (h q) -> p h q", q=512)[:, :, lo:512]
                    psv = ps[:, :].rearrange("p (h q) -> p h q", q=512)[:, :, lo:512]
                    nc.scalar.activation(exv, psv, AF.Exp, scale=es_ap)
                    if jl >= 0:
                        # diagonal k-tile: only the q-slice qi == jl straddles the
                        # causal boundary (qi < jl slices are never read by PV,
                        # qi > jl slices are fully valid) -> one triangle-mask
                        # multiply covering both head halves
                        exd = ex[:, :].rearrange("p (h q) -> p h q", q=512)[
                            :, :, jl * 128 : jl * 128 + 128
                        ]
                        nc.vector.tensor_tensor(
                            exd,
                            exd,
                            maskt[:, :]
                            .rearrange("p (o f) -> p o f", o=1)
                            .broadcast_to([128, 2, 128]),
                            OP.mult,
                        )
                    ex_tiles.append(ex)
                for hh in range(2):
                    h = 2 * hp + hh
                    pvp4 = psV.tile([128, 4 * (HD + 1)], F32, tag="pv", name="pv_ps")
                    for qi in range(4):
                        qig = 4 * qb + qi
                        for j in range(qig + 1):
                            nc.tensor.matmul(
                                pvp4[:, qi * (HD + 1) : (qi + 1) * (HD + 1)],
                                ex_tiles[j][
                                    :, hh * 512 + qi * 128 : hh * 512 + qi * 128 + 128
                                ],
                                Vaug[j][:, h * (HD + 1) : (h + 1) * (HD + 1)],
                                start=(j == 0),
                                stop=(j == qig),
                            )
                    pv4v = pvp4[:, :].rearrange("p (q c) -> p q c", c=HD + 1)
                    rcp4 = rcpp.tile([128, 4], F32, tag="rcp", name="rcp4")
                    nc.vector.reciprocal(rcp4[:, :], pv4v[:, :, HD])
                    rcp4s = rcpp.tile([128, 4], F32, tag="rcp", name="rcp4s")
                    nc.vector.tensor_scalar(
                        rcp4s[:, :], rcp4[:, :], sc_ap, None, OP.mult
                    )
                    dstv = attnN[qb][:, :].rearrange("p (q c) -> p q c", c=CH)[
                        :, :, h * HD : (h + 1) * HD
                    ]
                    nc.vector.tensor_tensor(
                        dstv,
                        pv4v[:, :, 0:HD],
                        rcp4s[:, :]
                        .rearrange("p (q o) -> p q o", o=1)
                        .broadcast_to([128, 4, HD]),
                        OP.mult,
                    )
            # ---- transpose attention output into attnT[cb] [128(ch), T] ----
            for qp in range(2):  # pairs of q-tiles
                tp4 = psV.tile([128, 512], F16, tag="pv", name="tp4")
                for ti in range(2):
                    qi = 2 * qp + ti
                    for cb in range(2):
                        nc.tensor.transpose(
                            tp4[:, (2 * ti + cb) * 128 : (2 * ti + cb) * 128 + 128],
                            attnN[qb][:, qi * CH + cb * 128 : qi * CH + (cb + 1) * 128],
                            id16[:, :],
                        )
                tt0 = 4 * qb + 2 * qp
                dstv = (
                    attnTa[:, :]
                    .rearrange("p (cb t) -> p cb t", cb=2)[
                        :, :, tt0 * 128 : (tt0 + 2) * 128
                    ]
                    .rearrange("p cb (ti f) -> p ti cb f", f=128)
                )
                srcv = tp4[:, :].rearrange("p (ti cb f) -> p ti cb f", ti=2, cb=2)
                nc.vector.tensor_copy(dstv, srcv)
            # ---- LpT block ----
            if use_lora_proj:
                ps = psB.tile([R, 512], F32, tag="mm", name="lp_ps")
                for cb in range(2):
                    nc.tensor.matmul(
                        ps[:, :],
                        ApT[cb][:, :],
                        attnT[cb][:, qb * 512 : (qb + 1) * 512],
                        start=(cb == 0),
                        stop=(cb == 1),
                    )
                nc.scalar.mul(LpT[:, qb * 512 : (qb + 1) * 512], ps[:, :], ALPHA_OVER_R)
            # ---- proj partial for this q-block ----
            for qi in range(4):
                tt = 4 * qb + qi
                po_t = outp.tile([128, C], F16, tag="po", name=f"po{tt}")
                ps2s = [
                    psB.tile([128, 512], F32, tag="mm", name=f"pj_ps{nb}")
                    for nb in range(2)
                ]
                for cb in range(2):
                    for nb in range(2):
                        nc.tensor.matmul(
                            ps2s[nb][:, :],
                            attnT[cb][:, tt * 128 : (tt + 1) * 128],
                            wpT[cb][:, nb * 512 : (nb + 1) * 512],
                            start=(cb == 0),
                            stop=(cb == 1 and not use_lora_proj),
                        )
                if use_lora_proj:
                    for nb in range(2):
                        nc.tensor.matmul(
                            ps2s[nb][:, :],
                            LpT[:, tt * 128 : (tt + 1) * 128],
                            BpT[:, nb * 512 : (nb + 1) * 512],
                            start=False,
                            stop=True,
                        )
                for nb in range(2):
                    nc.vector.tensor_copy(po_t[:, nb * 512 : (nb + 1) * 512], ps2s[nb][:, :])
                nc.sync.dma_start(d["out"][tt * 128 : (tt + 1) * 128, :], po_t[:, :])


def _build_program(use_bias, use_lora_attn, use_lora_proj):
    nc = bacc.Bacc("TRN2", target_bir_lowering=False, debug=False, num_devices=N_CORES)

    def din(name, shape, dt=F16):
        return nc.dram_tensor(name, shape, dt, kind="ExternalInput").ap()

    d = {
        "xT": din("xT", [C, T]),
        "wqkT": din("wqkT", [C, 2 * CH]),
        "wvT": din("wvT", [C, CH]),
        "wpT": din("wpT", [CH, C]),
        "aatT": din("aatT", [C, R]),
        "bqkT": din("bqkT", [R, 2 * CH]),
        "bvT": din("bvT", [R, CH]),
        "apT": din("apT", [CH, R]),
        "bpT": din("bpT", [R, C]),
        "bqk": din("bqk", [1, 2 * CH]),
        "bv": din("bv", [1, CH]),
        "consts": din("consts", [128, 4], F32),
        "id16": din("id16", [128, 128]),
        "masks": din("masks", [128, 128]),
        "out": nc.dram_tensor("out", [T, C], F16, kind="ExternalOutput").ap(),
    }
    with tile.TileContext(nc) as tc:
        _build_body(nc, tc, d, use_bias, use_lora_attn, use_lora_proj)
    nc.compile()
    _dedupe_ldweights(nc)
    return nc


def _ap_key(ap):
    try:
        t = ap.tensor_name if hasattr(ap, "tensor_name") else None
        return (t, str(ap))
    except Exception:
        return (None, object())


def _dedupe_ldweights(nc):
    """Remove back-to-back InstLdweights that reload identical weights.

    Safe only when the duplicate has no semaphore waits/updates (any guarded
    rewrite of the weight region would carry its wait on the ldweights) and
    the intervening instruction is a single non-transpose matmul."""
    removed = 0
    pe = mybir.EngineType.PE
    for blk in nc.m.functions[0].blocks:
        insts = blk.instructions
        keep = []
        prev_key = None
        for inst in insts:
            if getattr(inst, "engine", None) != pe:
                keep.append(inst)
                continue
            t = type(inst).__name__
            if t == "InstLdweights":
                si = inst.sync_info
                clean = si is None or (not si.on_wait and not si.on_update)
                key = str(inst.ins[0])
                if clean and prev_key is not None and key == prev_key:
                    removed += 1
                    continue
                prev_key = key
            elif t == "InstMatmult":
                if getattr(inst, "is_transpose", False):
                    prev_key = None
            keep.append(inst)
        if len(keep) != len(insts):
            blk.instructions = keep
    return removed


_CACHE = {}


def get_program(use_bias=True, use_lora_attn=True, use_lora_proj=True):
    key = (use_bias, use_lora_attn, use_lora_proj)
    if key not in _CACHE:
        _CACHE[key] = _build_program(*key)
    return _CACHE[key]


def make_in_maps(
    hidden_states, W_attn, b_attn, A_attn, B_attn, W_proj, b_proj, A_proj, B_proj,
    kv_scale, kv_zp,
):
    f32, f16 = np.float32, np.float16
    hidden_states = np.asarray(hidden_states, f32)
    W_attn = np.asarray(W_attn, f32)
    b_attn = np.asarray(b_attn, f32)
    A_attn = np.asarray(A_attn, f32)
    B_attn = np.asarray(B_attn, f32)
    W_proj = np.asarray(W_proj, f32)
    A_proj = np.asarray(A_proj, f32)
    B_proj = np.asarray(B_proj, f32)
    scale = f32(np.asarray(kv_scale, f32).reshape(-1)[0])
    zp = f32(np.asarray(kv_zp, f32).reshape(-1)[0])

    consts = np.zeros((128, 4), f32)
    consts[:, 0] = f32(1.0) / scale
    consts[:, 1] = zp
    consts[:, 2] = scale
    consts[:, 3] = np.float32(0.125) * scale

    # id16 + causal masks
    id16 = np.eye(128, dtype=f16)
    iota_p = np.arange(128)[:, None]
    iota_f = np.arange(512)[None, :]
    masks = (iota_f[:, :128] - iota_p >= 0).astype(f16)  # [128,128] lower=0 triangle

    ct = lambda a: np.ascontiguousarray(a).astype(f16)
    xTs = [ct(hidden_states[b].T) for b in range(B)]
    bpT = ct(B_proj.T)

    in_maps = []
    for c in range(N_CORES):
        b = c // 4
        hg = c % 4
        qs = slice(hg * CH, (hg + 1) * CH)
        ks = slice(C + hg * CH, C + (hg + 1) * CH)
        vs = slice(2 * C + hg * CH, 2 * C + (hg + 1) * CH)
        wqk = np.concatenate([W_attn[qs], W_attn[ks]], axis=0)
        bqkl = np.concatenate([B_attn[qs], B_attn[ks]], axis=0)
        in_maps.append(
            {
                "xT": xTs[b],
                "wqkT": ct(wqk.T),
                "wvT": ct(W_attn[vs].T),
                "wpT": ct(W_proj[:, hg * CH : (hg + 1) * CH].T),
                "aatT": ct(A_attn.T),
                "bqkT": ct(bqkl.T),
                "bvT": ct(B_attn[vs].T),
                "apT": ct(A_proj[:, hg * CH : (hg + 1) * CH].T),
                "bpT": bpT,
                "bqk": ct(np.concatenate([b_attn[qs], b_attn[ks]])[None, :]),
                "bv": ct(b_attn[vs][None, :]),
                "consts": consts,
                "id16": id16,
                "masks": masks,
            }
        )
    return in_maps


def variant_flags(b_attn, B_attn, B_proj):
    return (
        bool(np.any(np.asarray(b_attn))),
        bool(np.any(np.asarray(B_attn))),
        bool(np.any(np.asarray(B_proj))),
    )


def assemble_output(results, b_proj):
    out = np.zeros((B, T, C), np.float32)
    for c in range(N_CORES):
        out[c // 4] += results[c]["out"].astype(np.float32)
    out += np.asarray(b_proj, np.float32)[None, None, :]
    return out


def kernel(**inputs):
    flags = variant_flags(inputs["b_attn"], inputs["B_attn"], inputs["B_proj"])
    nc = get_program(*flags)
    in_maps = make_in_maps(**inputs)
    res = run_bass_kernel_spmd(nc, in_maps, core_ids=list(range(N_CORES)))
    return assemble_output(res.results, inputs["b_proj"])
